# revision 11
# baseline (speedup 1.0000x reference)
"""ChatGLM3 attention (B=2, S=2048, H=4096, 32 q-heads / 2 kv-heads, D=128)
on 8 Trainium2 NeuronCores.

Sharding: core c = 4*b + tp  (b in {0,1} data-parallel over batch,
tp in {0..3} tensor-parallel over heads). Each core computes the QKV
projection for its 8 q-heads + its kv head (k and v columns), applies RoPE,
and runs causal GQA attention for its 8 heads over the full sequence.
No collectives; per-core inputs/outputs are sharded and assembled on host.

Device layout is "transposed": qkvT [n, s] with head-dim on partitions, so
the projection needs no transposes (lhsT = w columns, rhs = hiddenT) and
scores come out as scoresT [k, s_q].

fp16 datapath: inputs are cast to fp16 on host (halves DMA traffic, and
fp16 matmuls run at 1 cycle/row at any tile width, enabling diagonal
column-slicing). Accumulation stays in fp32 PSUM. Weights are resident in
SBUF (loaded once per iteration). The causal mask is applied by a DVE
multiply with a constant 128x128 triangle on diagonal tiles (no PE mask
matmuls), and the softmax denominator is built by DVE accumulation of the
prob tiles plus a single ones-matmul per (head, q-chunk).
"""
import numpy as np
from contextlib import ExitStack

import concourse.bacc as bacc
import concourse.tile as tile
import concourse.mybir as mybir
from concourse.bass import broadcast_tensor_aps

# Problem constants (hardcoded per contract)
B, S, HIDDEN = 2, 2048, 4096
NUM_HEADS, NUM_KV_HEADS, D = 32, 2, 128
ROPE_BASE = 10000.0
N_CORES = 8
HEADS_PER_CORE = NUM_HEADS // 4          # 8 (TP=4)
NC_CHUNKS = HEADS_PER_CORE + 2           # 8 q + 1 k + 1 v = 10 n-chunks of 128
SC = 512                                 # s-chunk (psum bank = 512 fp32)
NSC = S // SC                            # 4
NKT = S // 128                           # 16 k-tiles
KCH = HIDDEN // 128                      # 32 contraction chunks
SCALE = float(D) ** -0.5

f32 = mybir.dt.float32
f32r = mybir.dt.float32r
f16 = mybir.dt.float16

_CACHE: dict = {}


def _build_nc(loop_n: int = 1):
    nc = bacc.Bacc(trn_type="TRN2", target_bir_lowering=False, debug=False)

    hT_d = nc.dram_tensor("hT", [HIDDEN, S], f16, kind="ExternalInput").ap()
    # wc pre-tiled on host: wc_t[n*128+p, k*128+m] = w_slice[k*128+p, n*128+m]
    wc_d = nc.dram_tensor("wc", [NC_CHUNKS * 128, HIDDEN], f16, kind="ExternalInput").ap()
    cosF_d = nc.dram_tensor("cosF", [128, S], f16, kind="ExternalInput").ap()
    sinS_d = nc.dram_tensor("sinS", [128, S], f16, kind="ExternalInput").ap()
    ident_d = nc.dram_tensor("ident", [128, 128], f16, kind="ExternalInput").ap()
    perm_d = nc.dram_tensor("perm", [128, 128], f16, kind="ExternalInput").ap()
    trimask_d = nc.dram_tensor("trimask", [128, 128], f16, kind="ExternalInput").ap()
    onesc_d = nc.dram_tensor("onesc", [128, 1], f16, kind="ExternalInput").ap()
    outT_d = nc.dram_tensor("outT", [HEADS_PER_CORE * 128, S], f16, kind="ExternalOutput").ap()

    with tile.TileContext(nc) as tc, ExitStack() as ctx:
        if loop_n > 1:
            ctx.enter_context(tc.For_i(0, loop_n, 1))
        cpool = ctx.enter_context(tc.tile_pool(name="consts", bufs=1))
        w_pool = ctx.enter_context(tc.tile_pool(name="wsb", bufs=1))
        qk_pool = ctx.enter_context(tc.tile_pool(name="qkt", bufs=1))
        v_pool = ctx.enter_context(tc.tile_pool(name="vsb", bufs=1))

        ident = cpool.tile([128, 128], f16, tag="ident")
        perm = cpool.tile([128, 128], f16, tag="perm")
        trimask = cpool.tile([128, 128], f16, tag="trimask")
        ones_c = cpool.tile([128, 1], f16, tag="onesc")
        nc.sync.dma_start(ident[:], ident_d)
        nc.sync.dma_start(perm[:], perm_d)
        nc.sync.dma_start(trimask[:], trimask_d)
        nc.sync.dma_start(ones_c[:], onesc_d)

        # weights resident in SBUF for the whole iteration (fp16, 80KB/part);
        # wc0 is DMA'd first so the first projection chunk can start ASAP —
        # the rest are issued after sc=0's hT tiles (see below)
        wc_sb = [w_pool.tile([128, HIDDEN], f16, tag=f"wc{n}", name=f"wc{n}")
                 for n in range(NC_CHUNKS)]
        nc.sync.dma_start(wc_sb[0][:], wc_d[0:128, :])

        # persistent: 8 q heads + k, all RoPE'd, [d, s] layout
        qkT = [qk_pool.tile([128, S], f16, tag=f"qkT{n}", name=f"qkT{n}")
               for n in range(9)]
        # v in [s, d] layout: tile t at columns t*128:(t+1)*128
        v_sb = v_pool.tile([128, NKT * 128], f16, tag="vsb")

        ht_pool = ctx.enter_context(tc.tile_pool(name="hts", bufs=1))
        tab_pool = ctx.enter_context(tc.tile_pool(name="tabs", bufs=2))
        rope_pool = ctx.enter_context(tc.tile_pool(name="rope", bufs=2))
        probs_pool = ctx.enter_context(tc.tile_pool(name="probs", bufs=4))
        acc_pool = ctx.enter_context(tc.tile_pool(name="accp", bufs=2))
        att_pool = ctx.enter_context(tc.tile_pool(name="att", bufs=2))
        pp = ctx.enter_context(tc.tile_pool(name="pp", bufs=2, space="PSUM"))
        scp = ctx.enter_context(tc.tile_pool(name="scp", bufs=2, space="PSUM"))
        pvl = ctx.enter_context(tc.tile_pool(name="pvl", bufs=2, space="PSUM"))
        mps = ctx.enter_context(tc.tile_pool(name="mps", bufs=2, space="PSUM"))

        kT = qkT[8]
        Exp = mybir.ActivationFunctionType.Exp
        for sc in range(NSC):
            ssl = slice(sc * SC, (sc + 1) * SC)
            # ---- projection pass for this s-chunk ----
            ht = ht_pool.tile([128, KCH * SC], f16, tag="ht")
            for k in range(KCH):
                nc.sync.dma_start(
                    ht[:, k * SC:(k + 1) * SC], hT_d[k * 128:(k + 1) * 128, ssl])
            cos_t = tab_pool.tile([128, SC], f16, tag="cos")
            sin_t = tab_pool.tile([128, SC], f16, tag="sin")
            nc.sync.dma_start(cos_t[:], cosF_d[:, ssl])
            nc.sync.dma_start(sin_t[:], sinS_d[:, ssl])
            if sc == 0:
                for n in range(1, NC_CHUNKS):
                    nc.sync.dma_start(wc_sb[n][:], wc_d[n * 128:(n + 1) * 128, :])

            for n in range(NC_CHUNKS):
                psum = pp.tile([128, SC], f32, tag="proj")
                for k in range(KCH):
                    nc.tensor.matmul(
                        psum[:], wc_sb[n][:, k * 128:(k + 1) * 128],
                        ht[:, k * SC:(k + 1) * SC],
                        start=(k == 0), stop=(k == KCH - 1),
                    )
                with nc.allow_low_precision(reason="fp16 datapath"):
                    if n < 9:
                        # RoPE: out = raw*cos + swap(raw)*sin_signed
                        qraw = rope_pool.tile([128, SC], f16, tag="qraw")
                        nc.scalar.copy(qraw[:], psum[:])
                        swps = mps.tile([128, SC], f32, tag="mps", name="swps")
                        nc.tensor.matmul(swps[:], perm[:], qraw[:], start=True, stop=True)
                        t1 = rope_pool.tile([128, SC], f16, tag="t1")
                        nc.vector.tensor_mul(t1[:], qraw[:], cos_t[:])
                        t2 = rope_pool.tile([128, SC], f16, tag="t2")
                        nc.vector.tensor_mul(t2[:], swps[:], sin_t[:])
                        nc.vector.tensor_add(qkT[n][:, ssl], t1[:], t2[:])
                    else:
                        vraw = rope_pool.tile([128, SC], f16, tag="qraw", name="vraw")
                        nc.scalar.copy(vraw[:], psum[:])
                        for j in range(SC // 128):
                            vt = mps.tile([128, 128], f16, tag="mps", name="vt")
                            nc.tensor.transpose(
                                vt[:], vraw[:, j * 128:(j + 1) * 128], ident[:])
                            kt_glob = sc * (SC // 128) + j
                            nc.vector.tensor_copy(
                                v_sb[:, kt_glob * 128:(kt_glob + 1) * 128], vt[:])

            # ---- attention for q-chunk qc == sc (all heads) ----
            qc = sc
            n_kt = (qc + 1) * (SC // 128)
            for h in range(HEADS_PER_CORE):
                pv = pvl.tile([128, SC], f32, tag="pv")
                acc = acc_pool.tile([128, SC], f16, tag="acc")
                with nc.allow_low_precision(reason="fp16 attention datapath"):
                    for kt in range(n_kt):
                        j = kt - 4 * qc
                        c0 = 128 * j if j >= 0 else 0   # diag tiles: skip masked cols
                        w = SC - c0
                        scps = scp.tile([128, SC], f32, tag="sc")
                        nc.tensor.matmul(
                            scps[:, :w], kT[:, kt * 128:(kt + 1) * 128],
                            qkT[h][:, qc * SC + c0:(qc + 1) * SC],
                            start=True, stop=True,
                        )
                        if kt == 0:
                            probs = acc   # exp(kt=0) initializes the accumulator
                        else:
                            probs = probs_pool.tile([128, SC], f16, tag="probs")
                        nc.scalar.activation(probs[:, c0:], scps[:, :w], Exp, scale=SCALE)
                        if j >= 0:
                            # zero the strict lower triangle of the leading
                            # 128-col block (q < k region of the diagonal tile)
                            nc.vector.tensor_mul(
                                probs[:, c0:c0 + 128], probs[:, c0:c0 + 128], trimask[:])
                        nc.tensor.matmul(
                            pv[:, c0:], v_sb[:, kt * 128:(kt + 1) * 128], probs[:, c0:],
                            start=(kt == 0), stop=(kt == n_kt - 1),
                        )
                        if kt > 0:
                            nc.vector.tensor_add(acc[:, c0:], acc[:, c0:], probs[:, c0:])
                    lacc_ps = mps.tile([1, SC], f32, tag="mps", name="lacc")
                    nc.tensor.matmul(lacc_ps[:], ones_c[:], acc[:], start=True, stop=True)
                    lrec = att_pool.tile([1, SC], f32r, tag="lrec")
                    nc.vector.reciprocal(lrec[:], lacc_ps[:])
                    lexp = att_pool.tile([128, SC], f32r, tag="lexpsb")
                    nc.gpsimd.partition_broadcast(lexp[:], lrec[:])
                    outn = att_pool.tile([128, SC], f16, tag="outn")
                    nc.vector.tensor_mul(outn[:], pv[:], lexp[:])
                nc.sync.dma_start(
                    outT_d[h * 128:(h + 1) * 128, qc * SC:(qc + 1) * SC], outn[:])

    nc.finalize()
    return nc


def _get_runner(loop_n: int = 1):
    """Build nc once and a cached jitted shard_map callable (axon/PJRT)."""
    key = f"runner{loop_n}"
    if key in _CACHE:
        return _CACHE[key]

    import jax
    import jax.numpy as jnp  # noqa: F401
    from jax.sharding import Mesh, PartitionSpec
    from jax.experimental.shard_map import shard_map
    from concourse.bass2jax import (
        install_neuronx_cc_hook, _bass_exec_p, partition_id_tensor,
    )
    import concourse.mybir as _mybir

    nc = _build_nc(loop_n)
    install_neuronx_cc_hook()

    partition_name = nc.partition_id_tensor.name if nc.partition_id_tensor else None
    in_names, out_names, out_avals, zero_outs = [], [], [], []
    for alloc in nc.m.functions[0].allocations:
        if not isinstance(alloc, _mybir.MemoryLocationSet):
            continue
        name = alloc.memorylocations[0].name
        if alloc.kind == "ExternalInput":
            if name != partition_name:
                in_names.append(name)
        elif alloc.kind == "ExternalOutput":
            shape = tuple(alloc.tensor_shape)
            npdt = _mybir.dt.np(alloc.dtype)
            out_avals.append(jax.core.ShapedArray(shape, npdt))
            out_names.append(name)
            zero_outs.append(np.zeros(shape, npdt))

    n_params = len(in_names)
    n_outs = len(out_avals)
    all_in_names = in_names + out_names
    if partition_name is not None:
        all_in_names.append(partition_name)
    donate = tuple(range(n_params, n_params + n_outs))

    def _body(*args):
        operands = list(args)
        if partition_name is not None:
            operands.append(partition_id_tensor())
        outs = _bass_exec_p.bind(
            *operands,
            out_avals=tuple(out_avals),
            in_names=tuple(all_in_names),
            out_names=tuple(out_names),
            lowering_input_output_aliases=(),
            sim_require_finite=True,
            sim_require_nnan=True,
            nc=nc,
        )
        return tuple(outs)

    devices = jax.devices()[:N_CORES]
    mesh = Mesh(np.asarray(devices), ("core",))
    in_specs = (PartitionSpec("core"),) * (n_params + n_outs)
    out_specs = (PartitionSpec("core"),) * n_outs
    fn = jax.jit(
        shard_map(_body, mesh=mesh, in_specs=in_specs, out_specs=out_specs,
                  check_rep=False),
        donate_argnums=donate,
        keep_unused=True,
    )

    runner = (fn, in_names, out_names, out_avals, zero_outs)
    _CACHE[key] = runner
    return runner


def _host_prep(positions, hidden_states, w_qkv):
    """Build the per-core input maps (shard + layout prep, no reference math)."""
    positions = np.asarray(positions)
    hidden_states = np.asarray(hidden_states, dtype=np.float32)
    w_qkv = np.asarray(w_qkv, dtype=np.float32)

    half = D // 2
    inv_freq = 1.0 / (ROPE_BASE ** (np.arange(half, dtype=np.float32) / half))
    ang = positions.astype(np.float32)[:, None] * inv_freq[None, :]  # [S, 64]
    cos = np.cos(ang)  # [S, 64]
    sin = np.sin(ang)
    cosF = np.empty((128, S), np.float16)
    sinS = np.empty((128, S), np.float16)
    cosF[:half] = cos.T
    cosF[half:] = cos.T
    sinS[:half] = -sin.T
    sinS[half:] = sin.T

    ident = np.eye(128, dtype=np.float16)
    perm = np.roll(np.eye(128, dtype=np.float16), 64, axis=0)
    trimask = np.triu(np.ones((128, 128), np.float16))
    onesc = np.ones((128, 1), np.float16)
    onesr = np.ones((1, 128), np.float32)

    hT = [np.ascontiguousarray(hidden_states[b].T.astype(np.float16))
          for b in range(B)]

    q_sz = NUM_HEADS * D
    in_maps = []
    for c in range(N_CORES):
        b, tp = divmod(c, 4)
        kv = tp // 2
        wq = w_qkv[:, tp * 1024:(tp + 1) * 1024]
        wk = w_qkv[:, q_sz + kv * 128: q_sz + (kv + 1) * 128]
        wv = w_qkv[:, q_sz + NUM_KV_HEADS * D + kv * 128:
                      q_sz + NUM_KV_HEADS * D + (kv + 1) * 128]
        wc = np.concatenate([wq, wk, wv], axis=1)  # [4096, 1280]
        # tile to [n*128+p, k*128+m] = wc[k*128+p, n*128+m]
        wc_t = np.ascontiguousarray(
            wc.reshape(HIDDEN // 128, 128, NC_CHUNKS, 128)
            .transpose(2, 1, 0, 3)
            .reshape(NC_CHUNKS * 128, HIDDEN)
            .astype(np.float16)
        )
        in_maps.append({
            "hT": hT[b], "wc": wc_t, "cosF": cosF, "sinS": sinS,
            "ident": ident, "perm": perm, "trimask": trimask,
            "onesc": onesc, "onesr": onesr,
        })
    return in_maps


def run_device(in_maps):
    """Run the compiled kernel on 8 cores; returns list of per-core outputs."""
    fn, in_names, out_names, out_avals, zero_outs = _get_runner()
    per_core = [[np.asarray(m[nm]) for nm in in_names] for m in in_maps]
    concat_in = [
        np.concatenate([per_core[c][i] for c in range(N_CORES)], axis=0)
        for i in range(len(in_names))
    ]
    concat_zeros = [
        np.zeros((N_CORES * z.shape[0], *z.shape[1:]), z.dtype) for z in zero_outs
    ]
    out_arrs = fn(*concat_in, *concat_zeros)
    return [
        {
            nm: np.asarray(out_arrs[i]).reshape(N_CORES, *out_avals[i].shape)[c]
            for i, nm in enumerate(out_names)
        }
        for c in range(N_CORES)
    ]


def kernel(positions, hidden_states, w_qkv):
    in_maps = _host_prep(positions, hidden_states, w_qkv)
    results = run_device(in_maps)
    out = np.empty((B, S, NUM_HEADS * D), np.float32)
    for c in range(N_CORES):
        b, tp = divmod(c, 4)
        oT = results[c]["outT"].astype(np.float32).reshape(HEADS_PER_CORE, 128, S)
        out[b, :, tp * 1024:(tp + 1) * 1024] = (
            oT.transpose(2, 0, 1).reshape(S, HEADS_PER_CORE * 128)
        )
    return out


# revision 13
# speedup vs baseline: 1.2080x; 1.2080x over previous
"""ChatGLM3 attention (B=2, S=2048, H=4096, 32 q-heads / 2 kv-heads, D=128)
on 8 Trainium2 NeuronCores.

Sharding: core c = 4*b + tp  (b in {0,1} data-parallel over batch,
tp in {0..3} tensor-parallel over heads). Each core computes the QKV
projection for its 8 q-heads + its kv head (k and v columns), applies RoPE,
and runs causal GQA attention for its 8 heads over the full sequence.
No collectives; per-core inputs/outputs are sharded and assembled on host.

Device layout is "transposed": qkvT [n, s] with head-dim on partitions, so
the projection needs no transposes (lhsT = w columns, rhs = hiddenT) and
scores come out as scoresT [k, s_q].

fp16 datapath: inputs are cast to fp16 on host (halves DMA traffic, and
fp16 matmuls run at 1 cycle/row at any tile width, enabling diagonal
column-slicing). Accumulation stays in fp32 PSUM. Weights are resident in
SBUF (loaded once per iteration). The causal mask is applied by a DVE
multiply with a constant 128x128 triangle on diagonal tiles (no PE mask
matmuls), and the softmax denominator is built by DVE accumulation of the
prob tiles plus a single ones-matmul per (head, q-chunk).
"""
import numpy as np
from contextlib import ExitStack

import concourse.bacc as bacc
import concourse.tile as tile
import concourse.mybir as mybir
from concourse.bass import broadcast_tensor_aps

# Problem constants (hardcoded per contract)
B, S, HIDDEN = 2, 2048, 4096
NUM_HEADS, NUM_KV_HEADS, D = 32, 2, 128
ROPE_BASE = 10000.0
N_CORES = 8
HEADS_PER_CORE = NUM_HEADS // 4          # 8 (TP=4)
NC_CHUNKS = HEADS_PER_CORE + 2           # 8 q + 1 k + 1 v = 10 n-chunks of 128
SC = 512                                 # s-chunk (psum bank = 512 fp32)
NSC = S // SC                            # 4
NKT = S // 128                           # 16 k-tiles
KCH = HIDDEN // 128                      # 32 contraction chunks
SCALE = float(D) ** -0.5

f32 = mybir.dt.float32
f32r = mybir.dt.float32r
f16 = mybir.dt.float16

_CACHE: dict = {}


def _build_nc(loop_n: int = 1):
    nc = bacc.Bacc(trn_type="TRN2", target_bir_lowering=False, debug=False)

    hT_d = nc.dram_tensor("hT", [HIDDEN, S], f16, kind="ExternalInput").ap()
    # wc pre-tiled on host: wc_t[n*128+p, k*128+m] = w_slice[k*128+p, n*128+m]
    wc_d = nc.dram_tensor("wc", [NC_CHUNKS * 128, HIDDEN], f16, kind="ExternalInput").ap()
    cosF_d = nc.dram_tensor("cosF", [128, S], f16, kind="ExternalInput").ap()
    sinS_d = nc.dram_tensor("sinS", [128, S], f16, kind="ExternalInput").ap()
    ident_d = nc.dram_tensor("ident", [128, 128], f16, kind="ExternalInput").ap()
    perm_d = nc.dram_tensor("perm", [128, 128], f16, kind="ExternalInput").ap()
    trimask_d = nc.dram_tensor("trimask", [128, 128], f16, kind="ExternalInput").ap()
    onesc_d = nc.dram_tensor("onesc", [128, 1], f16, kind="ExternalInput").ap()
    outT_d = nc.dram_tensor("outT", [HEADS_PER_CORE * 128, S], f16, kind="ExternalOutput").ap()

    with tile.TileContext(nc) as tc, ExitStack() as ctx:
        if loop_n > 1:
            ctx.enter_context(tc.For_i(0, loop_n, 1))
        cpool = ctx.enter_context(tc.tile_pool(name="consts", bufs=1))
        w_pool = ctx.enter_context(tc.tile_pool(name="wsb", bufs=1))
        qk_pool = ctx.enter_context(tc.tile_pool(name="qkt", bufs=1))
        v_pool = ctx.enter_context(tc.tile_pool(name="vsb", bufs=1))

        ident = cpool.tile([128, 128], f16, tag="ident")
        perm = cpool.tile([128, 128], f16, tag="perm")
        trimask = cpool.tile([128, 128], f16, tag="trimask")
        ones_c = cpool.tile([128, 1], f16, tag="onesc")
        nc.sync.dma_start(ident[:], ident_d)
        nc.sync.dma_start(perm[:], perm_d)
        nc.sync.dma_start(trimask[:], trimask_d)
        nc.sync.dma_start(ones_c[:], onesc_d)

        # weights resident in SBUF for the whole iteration (fp16, 80KB/part);
        # wc0 is DMA'd first so the first projection chunk can start ASAP —
        # the rest are issued after sc=0's hT tiles (see below)
        wc_sb = [w_pool.tile([128, HIDDEN], f16, tag=f"wc{n}", name=f"wc{n}")
                 for n in range(NC_CHUNKS)]
        nc.sync.dma_start(wc_sb[0][:], wc_d[0:128, :])

        # persistent: 8 q heads + k, all RoPE'd, [d, s] layout
        qkT = [qk_pool.tile([128, S], f16, tag=f"qkT{n}", name=f"qkT{n}")
               for n in range(9)]
        # v in [s, d] layout: tile t at columns t*128:(t+1)*128
        v_sb = v_pool.tile([128, NKT * 128], f16, tag="vsb")

        ht_pool = ctx.enter_context(tc.tile_pool(name="hts", bufs=1))
        tab_pool = ctx.enter_context(tc.tile_pool(name="tabs", bufs=2))
        rope_pool = ctx.enter_context(tc.tile_pool(name="rope", bufs=2))
        probs_pool = ctx.enter_context(tc.tile_pool(name="probs", bufs=4))
        acc_pool = ctx.enter_context(tc.tile_pool(name="accp", bufs=2))
        att_pool = ctx.enter_context(tc.tile_pool(name="att", bufs=2))
        pp = ctx.enter_context(tc.tile_pool(name="pp", bufs=2, space="PSUM"))
        scp = ctx.enter_context(tc.tile_pool(name="scp", bufs=2, space="PSUM"))
        pvl = ctx.enter_context(tc.tile_pool(name="pvl", bufs=2, space="PSUM"))
        mps = ctx.enter_context(tc.tile_pool(name="mps", bufs=2, space="PSUM"))

        # chunk order in wc: n=0 -> k head, n=1 -> v head, n=2+h -> q head h
        kT = qkT[8]
        Exp = mybir.ActivationFunctionType.Exp
        for sc in range(NSC):
            ssl = slice(sc * SC, (sc + 1) * SC)
            ht = ht_pool.tile([128, KCH * SC], f16, tag="ht")
            for k in range(KCH):
                nc.sync.dma_start(
                    ht[:, k * SC:(k + 1) * SC], hT_d[k * 128:(k + 1) * 128, ssl])
            cos_t = tab_pool.tile([128, SC], f16, tag="cos")
            sin_t = tab_pool.tile([128, SC], f16, tag="sin")
            nc.sync.dma_start(cos_t[:], cosF_d[:, ssl])
            nc.sync.dma_start(sin_t[:], sinS_d[:, ssl])
            if sc == 0:
                for n in range(1, NC_CHUNKS):
                    nc.sync.dma_start(wc_sb[n][:], wc_d[n * 128:(n + 1) * 128, :])

            def proj_chunk(n):
                psum = pp.tile([128, SC], f32, tag="proj", name="proj")
                for k in range(KCH):
                    nc.tensor.matmul(
                        psum[:], wc_sb[n][:, k * 128:(k + 1) * 128],
                        ht[:, k * SC:(k + 1) * SC],
                        start=(k == 0), stop=(k == KCH - 1),
                    )
                with nc.allow_low_precision(reason="fp16 datapath"):
                    if n != 1:
                        # RoPE: out = raw*cos + swap(raw)*sin_signed
                        dst = kT if n == 0 else qkT[n - 2]
                        qraw = rope_pool.tile([128, SC], f16, tag="qraw", name="qraw")
                        nc.scalar.copy(qraw[:], psum[:])
                        swps = mps.tile([128, SC], f32, tag="mps", name="swps")
                        nc.tensor.matmul(swps[:], perm[:], qraw[:], start=True, stop=True)
                        t1 = rope_pool.tile([128, SC], f16, tag="t1", name="t1")
                        nc.vector.tensor_mul(t1[:], qraw[:], cos_t[:])
                        t2 = rope_pool.tile([128, SC], f16, tag="t2", name="t2")
                        nc.vector.tensor_mul(t2[:], swps[:], sin_t[:])
                        nc.vector.tensor_add(dst[:, ssl], t1[:], t2[:])
                    else:
                        vraw = rope_pool.tile([128, SC], f16, tag="qraw", name="vraw")
                        nc.scalar.copy(vraw[:], psum[:])
                        for j in range(SC // 128):
                            vt = mps.tile([128, 128], f16, tag="mps", name="vt")
                            nc.tensor.transpose(
                                vt[:], vraw[:, j * 128:(j + 1) * 128], ident[:])
                            kt_glob = sc * (SC // 128) + j
                            nc.vector.tensor_copy(
                                v_sb[:, kt_glob * 128:(kt_glob + 1) * 128], vt[:])

            qc = sc
            n_kt = (qc + 1) * (SC // 128)

            def attn_head(h):
                pv = pvl.tile([128, SC], f32, tag="pv", name="pv")
                acc = acc_pool.tile([128, SC], f16, tag="acc", name="acc")
                with nc.allow_low_precision(reason="fp16 attention datapath"):
                    for kt in range(n_kt):
                        j = kt - 4 * qc
                        c0 = 128 * j if j >= 0 else 0   # diag tiles: skip masked cols
                        w = SC - c0
                        scps = scp.tile([128, SC], f32, tag="sc", name="sc")
                        nc.tensor.matmul(
                            scps[:, :w], kT[:, kt * 128:(kt + 1) * 128],
                            qkT[h][:, qc * SC + c0:(qc + 1) * SC],
                            start=True, stop=True,
                        )
                        if kt == 0:
                            probs = acc   # exp(kt=0) initializes the accumulator
                        else:
                            probs = probs_pool.tile([128, SC], f16, tag="probs",
                                                    name="probs")
                        nc.scalar.activation(probs[:, c0:], scps[:, :w], Exp, scale=SCALE)
                        if j >= 0:
                            # zero the strict lower triangle of the leading
                            # 128-col block (q < k region of the diagonal tile)
                            nc.vector.tensor_mul(
                                probs[:, c0:c0 + 128], probs[:, c0:c0 + 128], trimask[:])
                        nc.tensor.matmul(
                            pv[:, c0:], v_sb[:, kt * 128:(kt + 1) * 128], probs[:, c0:],
                            start=(kt == 0), stop=(kt == n_kt - 1),
                        )
                        if kt > 0:
                            nc.vector.tensor_add(acc[:, c0:], acc[:, c0:], probs[:, c0:])
                    lacc_ps = mps.tile([1, SC], f32, tag="mps", name="lacc")
                    nc.tensor.matmul(lacc_ps[:], ones_c[:], acc[:], start=True, stop=True)
                    lrec = att_pool.tile([1, SC], f32r, tag="lrec", name="lrec")
                    nc.vector.reciprocal(lrec[:], lacc_ps[:])
                    lexp = att_pool.tile([128, SC], f32r, tag="lexpsb", name="lexp")
                    nc.gpsimd.partition_broadcast(lexp[:], lrec[:])
                    outn = att_pool.tile([128, SC], f16, tag="outn", name="outn")
                    nc.vector.tensor_mul(outn[:], pv[:], lexp[:])
                nc.sync.dma_start(
                    outT_d[h * 128:(h + 1) * 128, qc * SC:(qc + 1) * SC], outn[:])

            # k and v chunks first, then interleave q-head projection with the
            # previous head's attention so Act exp work overlaps PE proj bursts
            proj_chunk(0)
            proj_chunk(1)
            for h in range(HEADS_PER_CORE):
                proj_chunk(2 + h)
                if h >= 1:
                    attn_head(h - 1)
            attn_head(HEADS_PER_CORE - 1)

    nc.finalize()
    return nc


def _get_runner(loop_n: int = 1):
    """Build nc once and a cached jitted shard_map callable (axon/PJRT)."""
    key = f"runner{loop_n}"
    if key in _CACHE:
        return _CACHE[key]

    import jax
    import jax.numpy as jnp  # noqa: F401
    from jax.sharding import Mesh, PartitionSpec
    from jax.experimental.shard_map import shard_map
    from concourse.bass2jax import (
        install_neuronx_cc_hook, _bass_exec_p, partition_id_tensor,
    )
    import concourse.mybir as _mybir

    nc = _build_nc(loop_n)
    install_neuronx_cc_hook()

    partition_name = nc.partition_id_tensor.name if nc.partition_id_tensor else None
    in_names, out_names, out_avals, zero_outs = [], [], [], []
    for alloc in nc.m.functions[0].allocations:
        if not isinstance(alloc, _mybir.MemoryLocationSet):
            continue
        name = alloc.memorylocations[0].name
        if alloc.kind == "ExternalInput":
            if name != partition_name:
                in_names.append(name)
        elif alloc.kind == "ExternalOutput":
            shape = tuple(alloc.tensor_shape)
            npdt = _mybir.dt.np(alloc.dtype)
            out_avals.append(jax.core.ShapedArray(shape, npdt))
            out_names.append(name)
            zero_outs.append(np.zeros(shape, npdt))

    n_params = len(in_names)
    n_outs = len(out_avals)
    all_in_names = in_names + out_names
    if partition_name is not None:
        all_in_names.append(partition_name)
    donate = tuple(range(n_params, n_params + n_outs))

    def _body(*args):
        operands = list(args)
        if partition_name is not None:
            operands.append(partition_id_tensor())
        outs = _bass_exec_p.bind(
            *operands,
            out_avals=tuple(out_avals),
            in_names=tuple(all_in_names),
            out_names=tuple(out_names),
            lowering_input_output_aliases=(),
            sim_require_finite=True,
            sim_require_nnan=True,
            nc=nc,
        )
        return tuple(outs)

    devices = jax.devices()[:N_CORES]
    mesh = Mesh(np.asarray(devices), ("core",))
    in_specs = (PartitionSpec("core"),) * (n_params + n_outs)
    out_specs = (PartitionSpec("core"),) * n_outs
    fn = jax.jit(
        shard_map(_body, mesh=mesh, in_specs=in_specs, out_specs=out_specs,
                  check_rep=False),
        donate_argnums=donate,
        keep_unused=True,
    )

    runner = (fn, in_names, out_names, out_avals, zero_outs)
    _CACHE[key] = runner
    return runner


def _host_prep(positions, hidden_states, w_qkv):
    """Build the per-core input maps (shard + layout prep, no reference math)."""
    positions = np.asarray(positions)
    hidden_states = np.asarray(hidden_states, dtype=np.float32)
    w_qkv = np.asarray(w_qkv, dtype=np.float32)

    half = D // 2
    inv_freq = 1.0 / (ROPE_BASE ** (np.arange(half, dtype=np.float32) / half))
    ang = positions.astype(np.float32)[:, None] * inv_freq[None, :]  # [S, 64]
    cos = np.cos(ang)  # [S, 64]
    sin = np.sin(ang)
    cosF = np.empty((128, S), np.float16)
    sinS = np.empty((128, S), np.float16)
    cosF[:half] = cos.T
    cosF[half:] = cos.T
    sinS[:half] = -sin.T
    sinS[half:] = sin.T

    ident = np.eye(128, dtype=np.float16)
    perm = np.roll(np.eye(128, dtype=np.float16), 64, axis=0)
    trimask = np.triu(np.ones((128, 128), np.float16))
    onesc = np.ones((128, 1), np.float16)
    onesr = np.ones((1, 128), np.float32)

    hT = [np.ascontiguousarray(hidden_states[b].T.astype(np.float16))
          for b in range(B)]

    q_sz = NUM_HEADS * D
    in_maps = []
    for c in range(N_CORES):
        b, tp = divmod(c, 4)
        kv = tp // 2
        wq = w_qkv[:, tp * 1024:(tp + 1) * 1024]
        wk = w_qkv[:, q_sz + kv * 128: q_sz + (kv + 1) * 128]
        wv = w_qkv[:, q_sz + NUM_KV_HEADS * D + kv * 128:
                      q_sz + NUM_KV_HEADS * D + (kv + 1) * 128]
        wc = np.concatenate([wk, wv, wq], axis=1)  # [4096, 1280], k/v first
        # tile to [n*128+p, k*128+m] = wc[k*128+p, n*128+m]
        wc_t = np.ascontiguousarray(
            wc.reshape(HIDDEN // 128, 128, NC_CHUNKS, 128)
            .transpose(2, 1, 0, 3)
            .reshape(NC_CHUNKS * 128, HIDDEN)
            .astype(np.float16)
        )
        in_maps.append({
            "hT": hT[b], "wc": wc_t, "cosF": cosF, "sinS": sinS,
            "ident": ident, "perm": perm, "trimask": trimask,
            "onesc": onesc, "onesr": onesr,
        })
    return in_maps


def run_device(in_maps):
    """Run the compiled kernel on 8 cores; returns list of per-core outputs."""
    fn, in_names, out_names, out_avals, zero_outs = _get_runner()
    per_core = [[np.asarray(m[nm]) for nm in in_names] for m in in_maps]
    concat_in = [
        np.concatenate([per_core[c][i] for c in range(N_CORES)], axis=0)
        for i in range(len(in_names))
    ]
    concat_zeros = [
        np.zeros((N_CORES * z.shape[0], *z.shape[1:]), z.dtype) for z in zero_outs
    ]
    out_arrs = fn(*concat_in, *concat_zeros)
    return [
        {
            nm: np.asarray(out_arrs[i]).reshape(N_CORES, *out_avals[i].shape)[c]
            for i, nm in enumerate(out_names)
        }
        for c in range(N_CORES)
    ]


def kernel(positions, hidden_states, w_qkv):
    in_maps = _host_prep(positions, hidden_states, w_qkv)
    results = run_device(in_maps)
    out = np.empty((B, S, NUM_HEADS * D), np.float32)
    for c in range(N_CORES):
        b, tp = divmod(c, 4)
        oT = results[c]["outT"].astype(np.float32).reshape(HEADS_PER_CORE, 128, S)
        out[b, :, tp * 1024:(tp + 1) * 1024] = (
            oT.transpose(2, 0, 1).reshape(S, HEADS_PER_CORE * 128)
        )
    return out
